# revision 11
# baseline (speedup 1.0000x reference)
"""Trainium2 Bass kernel for nn_DiscriminatorBlock_38878043963811.

Strategy (v2)
-------------
Data-parallel over batch: 16 images -> 8 cores x 2 images. No collectives.

Algebraic restructuring (exact up to bf16, host-side folds):
  The ENTIRE linear path (fromrgb 1x1 + vertical & horizontal depthwise +
  low-rank residual + point 1x1) is one 3x3x3->512 conv = a single K=27
  matmul, M27 host-composed (clamp provably inactive, biases zero).

  TRANSPOSED execution: for each image row h, one matmul produces
  z^T[h] = [128 w, 512 c] (lhsT = shifted-sin patch [27, 128w] slice,
  rhs = M27 [27, 512c]).  PSUM is evacuated by ACT (Prelu, gain+FIR norms
  folded into scale) -- with a fraction of row-pairs offloaded to DVE as
  2-op prelu (max/scale + scalar_tensor_tensor) to balance engines.

  The whole 2x2-downsampling FIR ([1,3,3,1]x[1,3,3,1], stride 2) then runs
  on the PE as 4 tiny accumulating matmuls per output tile: for each
  (img, ho, ch-block): out2[128c, 64wo] += zT[2ho-1+a] (stationary) @
  (f_a * Fh) [128w, 64wo] (moving, only 64 columns -> 26.7ns each).
  out2 accumulates in PSUM f32, is evacuated to SBUF (DVE copy), then DMA'd.
"""

import sys

sys.path.insert(0, "/opt/trn_rl_repo")

import numpy as np
import ml_dtypes

import concourse.bass as bass
import concourse.bacc as bacc
import concourse.tile as tile
from concourse import mybir
from concourse.bass_utils import run_bass_kernel_spmd

f32 = mybir.dt.float32
bf16 = mybir.dt.bfloat16
AF = mybir.ActivationFunctionType
ALU = mybir.AluOpType

# ---- problem constants (hardcoded; kernel.py must be self-contained) ----
B, IMG_C, IN_C, OUT_C, S = 16, 3, 256, 512, 128
HIDDEN = IN_C
KGEN_IN = 32
KSIZE = 3
N_CORES = 8
B_LOC = B // N_CORES            # 2 images per core
GDW = np.float32(1.0 / np.sqrt(KSIZE))
ACT_SCALE = float(np.sqrt(2.0) / 64.0)   # lrelu gain * both FIR /8 norms

SFLAT = 130 * 130               # padded image, flattened with pitch 130
CH_Z = 32                       # z rows per s-chunk
SROWS = CH_Z + 2                # image rows held per chunk (3-tap halo)
CHW = SROWS * 130               # flat elems per s chunk

DVE_EVERY = 5                   # every 5th z row-pair prelu'd on DVE (~20%)

_CACHE = {}


def _sample_weight_np(grid, coeff, gauss_sigma, gauss_x, low_filter):
    """numpy port of reference._sample_weight (fp32)."""
    basis = np.sin(grid * np.float32(2.0 * np.pi)) * np.float32(np.exp(-0.5))
    w = coeff @ basis / np.float32(np.sqrt(HIDDEN))
    w = w - w.mean(dtype=np.float32)
    w = w * (1.0 / np.sqrt(np.mean(w * w, axis=0, keepdims=True, dtype=np.float32) + 1e-8))
    gs = 1.0 + gauss_sigma ** 2 / 5.0
    w = (w * np.exp(-(gauss_x ** 2) / (2.0 * gs))).astype(np.float32)
    nt = low_filter.shape[0]
    T = w.shape[1] - nt + 1
    out = np.empty((w.shape[0], T), np.float32)
    for t in range(T):
        out[:, t] = (w[:, t : t + nt] * low_filter[None, :]).sum(axis=1)
    return out[:, ::2]


def _build_program():
    nc = bacc.Bacc(None, target_bir_lowering=False)
    s_d = nc.declare_dram_parameter("s", [B_LOC, 27, SFLAT], bf16, isOutput=False)
    m27_d = nc.declare_dram_parameter("m27", [27, OUT_C], bf16, isOutput=False)
    fh_d = nc.declare_dram_parameter("fh", [128, 256], bf16, isOutput=False)
    out_d = nc.declare_dram_parameter("out", [B_LOC, OUT_C, 64, 64], bf16, isOutput=True)

    with tile.TileContext(nc) as tc:
        with (
            tc.tile_pool(name="const", bufs=1) as cpool,
            tc.tile_pool(name="spool", bufs=5) as spool,
            tc.tile_pool(name="zsb", bufs=6) as zpool,
            tc.tile_pool(name="qsc", bufs=2) as qpool,
            tc.tile_pool(name="osb", bufs=6) as opool,
            tc.tile_pool(name="zps", bufs=2, space="PSUM") as zps,
            tc.tile_pool(name="ops", bufs=1, space="PSUM") as ops,
        ):
            m27t = cpool.tile([27, OUT_C], bf16)
            nc.sync.dma_start(m27t[:], m27_d[:])
            fht = cpool.tile([128, 256], bf16)
            nc.sync.dma_start(fht[:], fh_d[:])

            pair_idx = 0  # global z row-pair counter (for DVE routing)

            for b in range(B_LOC):
                # s chunks for this image (4 chunks of 34 rows, stride 32)
                schunks = []
                for j in range(4):
                    st = spool.tile([27, CHW], bf16, tag="s", name=f"s_{b}_{j}")
                    nc.sync.dma_start(
                        st[:], s_d[b, :, CH_Z * 130 * j : CH_Z * 130 * j + CHW]
                    )
                    schunks.append(st)

                zrows = [None] * 32        # 4-row SBUF group tiles
                optiles = [None] * 4       # current octet's psum tile per cb

                def fir_ho(ho, b=b, zrows=zrows):
                    """FIR matmuls for one output row ho across 4 ch-blocks;
                    evac + DMA when an octet (8 ho) completes."""
                    o = ho // 8
                    taps = [(a, 2 * ho - 1 + a) for a in range(4)
                            if 0 <= 2 * ho - 1 + a <= 127]
                    for cb in range(4):
                        if ho % 8 == 0:
                            optiles[cb] = ops.tile([128, 512], f32,
                                                   tag=f"ops{cb}",
                                                   name=f"op_{b}_{o}_{cb}")
                        op = optiles[cb]
                        for i, (a, row) in enumerate(taps):
                            g2, sl = row // 4, row % 4
                            lhsT = zrows[g2][:, sl * 512 + cb * 128:
                                             sl * 512 + cb * 128 + 128]
                            nc.tensor.matmul(
                                op[:, (ho % 8) * 64 : (ho % 8) * 64 + 64],
                                lhsT, fht[:, a * 64 : a * 64 + 64],
                                start=(i == 0), stop=(i == len(taps) - 1),
                            )
                    if ho % 8 == 7:
                        for cb in range(4):
                            ot = opool.tile([128, 512], bf16, tag="osb",
                                            name=f"ot_{b}_{o}_{cb}")
                            nc.vector.tensor_copy(ot[:], optiles[cb][:])
                            nc.sync.dma_start(
                                out_d[b, cb * 128 : cb * 128 + 128,
                                      8 * o : 8 * o + 8, :],
                                ot[:].rearrange("p (r w) -> p r w", w=64),
                            )

                for g in range(32):       # 4-row z groups
                    zt = zpool.tile([128, 4 * 512], bf16, tag="zsb",
                                    name=f"z_{b}_{g}")
                    zrows[g] = zt
                    j = g // 8            # s chunk index
                    for half in range(2):
                        zp = zps.tile([128, 1024], f32, tag="zps",
                                      name=f"zp_{b}_{g}_{half}")
                        for r2 in range(2):
                            h = 4 * g + 2 * half + r2
                            lhsT = schunks[j][:, (h - CH_Z * j) * 130:
                                              (h - CH_Z * j) * 130 + 128]
                            nc.tensor.matmul(zp[:, r2 * 512 : r2 * 512 + 512],
                                             lhsT, m27t[:],
                                             start=True, stop=True)
                        dst = zt[:, half * 1024 : half * 1024 + 1024]
                        if pair_idx % DVE_EVERY == DVE_EVERY - 1:
                            # DVE prelu: s*(0.2x + 0.8 relu(x))
                            q = qpool.tile([128, 1024], bf16, tag="q",
                                           name=f"q_{b}_{g}_{half}")
                            nc.vector.tensor_scalar(
                                q[:], zp[:], 0.0, 0.8 * ACT_SCALE,
                                ALU.max, ALU.mult)
                            nc.vector.scalar_tensor_tensor(
                                dst, zp[:], 0.2 * ACT_SCALE, q[:],
                                ALU.mult, ALU.add)
                        else:
                            nc.scalar.activation(dst, zp[:], AF.Prelu,
                                                 bias=0.0, scale=ACT_SCALE,
                                                 alpha=0.2)
                        pair_idx += 1
                        # FIR drip: the one ho whose last z row (2ho+2)
                        # arrived with this pair (rows 4g+2h, 4g+2h+1)
                        ho = 2 * g + half - 1
                        if 0 <= ho <= 62:
                            fir_ho(ho)
                    if g == 31:
                        fir_ho(63)

    nc.compile()
    return nc


def kernel(**inputs):
    inputs = {k: np.asarray(v) for k, v in inputs.items()}
    img = inputs["img"].astype(np.float32)
    assert img.shape == (B, IMG_C, S, S)

    # ---- host-side weight generation (tiny) ----
    freqs = inputs["freqs"].astype(np.float32)
    phases = inputs["phases"].astype(np.float32)
    g = ((np.arange(KGEN_IN, dtype=np.float32) - (KGEN_IN - 1) / 2.0)
         * np.float32(2.0 / (KGEN_IN + 1)))
    gsig = np.float32(inputs["gauss_sigma"])
    gx = inputs["gauss_x"].astype(np.float32)
    lf = inputs["low_filter"].astype(np.float32)
    hz = _sample_weight_np(freqs[:, 0:1] * g[None, :] + phases[:, None],
                           inputs["hz_outdim"].astype(np.float32), gsig, gx, lf)
    vt = _sample_weight_np(freqs[:, 1:2] * g[None, :] + phases[:, None],
                           inputs["vt_outdim"].astype(np.float32), gsig, gx, lf)

    Wfr = inputs["fromrgb_w"][:, :, 0, 0].astype(np.float32) * np.float32(1.0 / np.sqrt(IMG_C))
    assert np.abs(Wfr).sum(1).max() < 250.0, "fromrgb clamp would be active"
    assert np.all(inputs["fromrgb_b"] == 0.0), "nonzero fromrgb bias unsupported"
    assert np.all(inputs["point_b"] == 0.0), "nonzero point bias unsupported"

    # k27[(d*3+jj)*3+r, c] = vt[c,d]*hz[c,jj]*GDW^2*Wfr[c,r]
    k9_np = np.zeros((27, IN_C), np.float32)
    for d in range(3):
        for r in range(3):
            for jj in range(3):
                k9_np[(d * 3 + jj) * 3 + r, :] = (
                    vt[:, d] * hz[:, jj] * GDW * GDW * Wfr[:, r]
                )
    L = inputs["lr_weight0"][:, :, 0, 0].astype(np.float32) * np.float32(1.0 / np.sqrt(IN_C))
    Pp = inputs["point_w"][:, :, 0, 0].astype(np.float32) * np.float32(1.0 / np.sqrt(IN_C))
    plw3 = (Pp @ L @ Wfr).T                      # [3, 512]
    m27_np = k9_np @ Pp.T                        # [27, 512]
    for r in range(3):
        m27_np[12 + r] += plw3[r]

    # FIR moving matrices: fh[w, a*64+wo] = f_a * f_b, b = w-(2wo-1)
    fir4 = np.array([1.0, 3.0, 3.0, 1.0], np.float32)
    fh_np = np.zeros((128, 256), np.float32)
    for a in range(4):
        for wo in range(64):
            for bb in range(4):
                w = 2 * wo - 1 + bb
                if 0 <= w <= 127:
                    fh_np[w, a * 64 + wo] = fir4[a] * fir4[bb]

    # shifted-sin flat layout: s27[b, (d*3+jj)*3+r, i] = spad[b, r, i + d*130 + jj]
    spad = np.zeros((B, IMG_C, 130, 130), np.float32)
    spad[:, :, 1:129, 1:129] = np.sin(img)
    spadf = np.zeros((B, IMG_C, SFLAT + 262), np.float32)
    spadf[:, :, :SFLAT] = spad.reshape(B, IMG_C, -1)
    s27_np = np.empty((B, 27, SFLAT), np.float32)
    for d in range(3):
        for jj in range(3):
            off = d * 130 + jj
            for r in range(3):
                s27_np[:, (d * 3 + jj) * 3 + r, :] = spadf[:, r, off : off + SFLAT]
    s27_np = s27_np.astype(ml_dtypes.bfloat16)

    shared = dict(
        m27=m27_np.astype(ml_dtypes.bfloat16),
        fh=fh_np.astype(ml_dtypes.bfloat16),
    )
    in_maps = [dict(s=np.ascontiguousarray(s27_np[c * B_LOC : (c + 1) * B_LOC]), **shared)
               for c in range(N_CORES)]

    if "nc" not in _CACHE:
        _CACHE["nc"] = _build_program()
    res = run_bass_kernel_spmd(_CACHE["nc"], in_maps, list(range(N_CORES)),
                               **_CACHE.get("run_kwargs", {}))
    _CACHE["last"] = res
    out = np.concatenate([np.asarray(res.results[c]["out"]) for c in range(N_CORES)],
                         axis=0)
    return out.astype(np.float32)


# revision 16
# speedup vs baseline: 1.2877x; 1.2877x over previous
"""Trainium2 Bass kernel for nn_DiscriminatorBlock_38878043963811.

Strategy (v2)
-------------
Data-parallel over batch: 16 images -> 8 cores x 2 images. No collectives.

Algebraic restructuring (exact up to bf16, host-side folds):
  The ENTIRE linear path (fromrgb 1x1 + vertical & horizontal depthwise +
  low-rank residual + point 1x1) is one 3x3x3->512 conv = a single K=27
  matmul, M27 host-composed (clamp provably inactive, biases zero).

  TRANSPOSED execution: for each image row h, one matmul produces
  z^T[h] = [128 w, 512 c] (lhsT = shifted-sin patch [27, 128w] slice,
  rhs = M27 [27, 512c]).  PSUM is evacuated by ACT (Prelu, gain+FIR norms
  folded into scale) -- with a fraction of row-pairs offloaded to DVE as
  2-op prelu (max/scale + scalar_tensor_tensor) to balance engines.

  The whole 2x2-downsampling FIR ([1,3,3,1]x[1,3,3,1], stride 2) then runs
  on the PE as 4 tiny accumulating matmuls per output tile: for each
  (img, ho, ch-block): out2[128c, 64wo] += zT[2ho-1+a] (stationary) @
  (f_a * Fh) [128w, 64wo] (moving, only 64 columns -> 26.7ns each).
  out2 accumulates in PSUM f32, is evacuated to SBUF (DVE copy), then DMA'd.
"""

import sys

sys.path.insert(0, "/opt/trn_rl_repo")

import numpy as np
import ml_dtypes

import concourse.bass as bass
import concourse.bacc as bacc
import concourse.tile as tile
from concourse import mybir
from concourse.bass_utils import run_bass_kernel_spmd

f32 = mybir.dt.float32
bf16 = mybir.dt.bfloat16
AF = mybir.ActivationFunctionType
ALU = mybir.AluOpType

# ---- problem constants (hardcoded; kernel.py must be self-contained) ----
B, IMG_C, IN_C, OUT_C, S = 16, 3, 256, 512, 128
HIDDEN = IN_C
KGEN_IN = 32
KSIZE = 3
N_CORES = 8
B_LOC = B // N_CORES            # 2 images per core
GDW = np.float32(1.0 / np.sqrt(KSIZE))
ACT_SCALE = float(np.sqrt(2.0) / 64.0)   # lrelu gain * both FIR /8 norms

SFLAT = 130 * 130               # padded image, flattened with pitch 130
CH_Z = 32                       # z rows per s-chunk
SROWS = CH_Z + 2                # image rows held per chunk (3-tap halo)
CHW = SROWS * 130               # flat elems per s chunk

DVE_EVERY = 5                   # every 5th z row-pair prelu'd on DVE (~20%)

_CACHE = {}


def _sample_weight_np(grid, coeff, gauss_sigma, gauss_x, low_filter):
    """numpy port of reference._sample_weight (fp32)."""
    basis = np.sin(grid * np.float32(2.0 * np.pi)) * np.float32(np.exp(-0.5))
    w = coeff @ basis / np.float32(np.sqrt(HIDDEN))
    w = w - w.mean(dtype=np.float32)
    w = w * (1.0 / np.sqrt(np.mean(w * w, axis=0, keepdims=True, dtype=np.float32) + 1e-8))
    gs = 1.0 + gauss_sigma ** 2 / 5.0
    w = (w * np.exp(-(gauss_x ** 2) / (2.0 * gs))).astype(np.float32)
    nt = low_filter.shape[0]
    T = w.shape[1] - nt + 1
    out = np.empty((w.shape[0], T), np.float32)
    for t in range(T):
        out[:, t] = (w[:, t : t + nt] * low_filter[None, :]).sum(axis=1)
    return out[:, ::2]


def _build_program():
    nc = bacc.Bacc(None, target_bir_lowering=False)
    s_d = nc.declare_dram_parameter("s", [B_LOC, 27, SFLAT], bf16, isOutput=False)
    m27_d = nc.declare_dram_parameter("m27", [27, OUT_C], bf16, isOutput=False)
    fh_d = nc.declare_dram_parameter("fh", [128, 256], bf16, isOutput=False)
    out_d = nc.declare_dram_parameter("out", [B_LOC, OUT_C, 64, 64], bf16, isOutput=True)

    with tile.TileContext(nc) as tc:
        with (
            tc.tile_pool(name="const", bufs=1) as cpool,
            tc.tile_pool(name="spool", bufs=5) as spool,
            tc.tile_pool(name="zsb", bufs=6) as zpool,
            tc.tile_pool(name="qsc", bufs=2) as qpool,
            tc.tile_pool(name="osb", bufs=6) as opool,
            tc.tile_pool(name="zps", bufs=3, space="PSUM") as zps,
            tc.tile_pool(name="ops", bufs=2, space="PSUM") as ops,
        ):
            m27t = cpool.tile([27, OUT_C], bf16)
            nc.sync.dma_start(m27t[:], m27_d[:])
            fht = cpool.tile([128, 256], bf16)
            nc.sync.dma_start(fht[:], fh_d[:])

            pair_idx = 0  # global z row-pair counter (for DVE routing)

            for b in range(B_LOC):
                # s chunks for this image (4 chunks of 34 rows, stride 32)
                schunks = []
                for j in range(4):
                    st = spool.tile([27, CHW], bf16, tag="s", name=f"s_{b}_{j}")
                    nc.sync.dma_start(
                        st[:], s_d[b, :, CH_Z * 130 * j : CH_Z * 130 * j + CHW]
                    )
                    schunks.append(st)

                zrows = [None] * 32        # 4-row SBUF group tiles
                pending = []               # ready FIR quanta (octet, cb)
                next_o = 0                 # next octet awaiting readiness

                def fir_quantum(o, cb, b=b, zrows=zrows):
                    """FIR matmuls for octet o (8 ho) x one ch-block; then
                    evac to SBUF bf16 + DMA."""
                    op = ops.tile([128, 512], f32, tag="ops",
                                  name=f"op_{b}_{o}_{cb}")
                    for ho in range(8 * o, 8 * o + 8):
                        taps = [(a, 2 * ho - 1 + a) for a in range(4)
                                if 0 <= 2 * ho - 1 + a <= 127]
                        for i, (a, row) in enumerate(taps):
                            g2, sl = row // 4, row % 4
                            lhsT = zrows[g2][:, sl * 512 + cb * 128:
                                             sl * 512 + cb * 128 + 128]
                            nc.tensor.matmul(
                                op[:, (ho % 8) * 64 : (ho % 8) * 64 + 64],
                                lhsT, fht[:, a * 64 : a * 64 + 64],
                                start=(i == 0), stop=(i == len(taps) - 1),
                            )
                    ot = opool.tile([128, 512], bf16, tag="osb",
                                    name=f"ot_{b}_{o}_{cb}")
                    nc.vector.tensor_copy(ot[:], op[:])
                    nc.sync.dma_start(
                        out_d[b, cb * 128 : cb * 128 + 128,
                              8 * o : 8 * o + 8, :],
                        ot[:].rearrange("p (r w) -> p r w", w=64),
                    )

                for g in range(32):       # 4-row z groups
                    zt = zpool.tile([128, 4 * 512], bf16, tag="zsb",
                                    name=f"z_{b}_{g}")
                    zrows[g] = zt
                    j = g // 8            # s chunk index
                    for half in range(2):
                        zp = zps.tile([128, 1024], f32, tag="zps",
                                      name=f"zp_{b}_{g}_{half}")
                        for r2 in range(2):
                            h = 4 * g + 2 * half + r2
                            lhsT = schunks[j][:, (h - CH_Z * j) * 130:
                                              (h - CH_Z * j) * 130 + 128]
                            nc.tensor.matmul(zp[:, r2 * 512 : r2 * 512 + 512],
                                             lhsT, m27t[:],
                                             start=True, stop=True)
                        dst = zt[:, half * 1024 : half * 1024 + 1024]
                        if pair_idx % DVE_EVERY == DVE_EVERY - 1:
                            # DVE prelu: s*(0.2x + 0.8 relu(x))
                            q = qpool.tile([128, 1024], bf16, tag="q",
                                           name=f"q_{b}_{g}_{half}")
                            nc.vector.tensor_scalar(
                                q[:], zp[:], 0.0, 0.8 * ACT_SCALE,
                                ALU.max, ALU.mult)
                            nc.vector.scalar_tensor_tensor(
                                dst, zp[:], 0.2 * ACT_SCALE, q[:],
                                ALU.mult, ALU.add)
                        else:
                            nc.scalar.activation(dst, zp[:], AF.Prelu,
                                                 bias=0.0, scale=ACT_SCALE,
                                                 alpha=0.2)
                        pair_idx += 1
                        # octet o's z window completes at row 16o+16
                        # (octet 7 at row 127 -- its row-128 tap is skipped)
                        rows_done = 4 * g + 2 * half + 1
                        if next_o <= 7 and rows_done >= (
                                16 * next_o + 16 if next_o < 7 else 127):
                            pending.extend((next_o, cb) for cb in range(4))
                            next_o += 1
                        if pending:
                            fir_quantum(*pending.pop(0))
                while pending:
                    fir_quantum(*pending.pop(0))

    nc.compile()
    return nc


def kernel(**inputs):
    inputs = {k: np.asarray(v) for k, v in inputs.items()}
    img = inputs["img"].astype(np.float32)
    assert img.shape == (B, IMG_C, S, S)

    # ---- host-side weight generation (tiny) ----
    freqs = inputs["freqs"].astype(np.float32)
    phases = inputs["phases"].astype(np.float32)
    g = ((np.arange(KGEN_IN, dtype=np.float32) - (KGEN_IN - 1) / 2.0)
         * np.float32(2.0 / (KGEN_IN + 1)))
    gsig = np.float32(inputs["gauss_sigma"])
    gx = inputs["gauss_x"].astype(np.float32)
    lf = inputs["low_filter"].astype(np.float32)
    hz = _sample_weight_np(freqs[:, 0:1] * g[None, :] + phases[:, None],
                           inputs["hz_outdim"].astype(np.float32), gsig, gx, lf)
    vt = _sample_weight_np(freqs[:, 1:2] * g[None, :] + phases[:, None],
                           inputs["vt_outdim"].astype(np.float32), gsig, gx, lf)

    Wfr = inputs["fromrgb_w"][:, :, 0, 0].astype(np.float32) * np.float32(1.0 / np.sqrt(IMG_C))
    assert np.abs(Wfr).sum(1).max() < 250.0, "fromrgb clamp would be active"
    assert np.all(inputs["fromrgb_b"] == 0.0), "nonzero fromrgb bias unsupported"
    assert np.all(inputs["point_b"] == 0.0), "nonzero point bias unsupported"

    # k27[(d*3+jj)*3+r, c] = vt[c,d]*hz[c,jj]*GDW^2*Wfr[c,r]
    k9_np = np.zeros((27, IN_C), np.float32)
    for d in range(3):
        for r in range(3):
            for jj in range(3):
                k9_np[(d * 3 + jj) * 3 + r, :] = (
                    vt[:, d] * hz[:, jj] * GDW * GDW * Wfr[:, r]
                )
    L = inputs["lr_weight0"][:, :, 0, 0].astype(np.float32) * np.float32(1.0 / np.sqrt(IN_C))
    Pp = inputs["point_w"][:, :, 0, 0].astype(np.float32) * np.float32(1.0 / np.sqrt(IN_C))
    plw3 = (Pp @ L @ Wfr).T                      # [3, 512]
    m27_np = k9_np @ Pp.T                        # [27, 512]
    for r in range(3):
        m27_np[12 + r] += plw3[r]

    # FIR moving matrices: fh[w, a*64+wo] = f_a * f_b, b = w-(2wo-1)
    fir4 = np.array([1.0, 3.0, 3.0, 1.0], np.float32)
    fh_np = np.zeros((128, 256), np.float32)
    for a in range(4):
        for wo in range(64):
            for bb in range(4):
                w = 2 * wo - 1 + bb
                if 0 <= w <= 127:
                    fh_np[w, a * 64 + wo] = fir4[a] * fir4[bb]

    # shifted-sin flat layout: s27[b, (d*3+jj)*3+r, i] = spad[b, r, i + d*130 + jj]
    spad = np.zeros((B, IMG_C, 130, 130), np.float32)
    spad[:, :, 1:129, 1:129] = np.sin(img)
    spadf = np.zeros((B, IMG_C, SFLAT + 262), np.float32)
    spadf[:, :, :SFLAT] = spad.reshape(B, IMG_C, -1)
    s27_np = np.empty((B, 27, SFLAT), np.float32)
    for d in range(3):
        for jj in range(3):
            off = d * 130 + jj
            for r in range(3):
                s27_np[:, (d * 3 + jj) * 3 + r, :] = spadf[:, r, off : off + SFLAT]
    s27_np = s27_np.astype(ml_dtypes.bfloat16)

    shared = dict(
        m27=m27_np.astype(ml_dtypes.bfloat16),
        fh=fh_np.astype(ml_dtypes.bfloat16),
    )
    in_maps = [dict(s=np.ascontiguousarray(s27_np[c * B_LOC : (c + 1) * B_LOC]), **shared)
               for c in range(N_CORES)]

    if "nc" not in _CACHE:
        _CACHE["nc"] = _build_program()
    res = run_bass_kernel_spmd(_CACHE["nc"], in_maps, list(range(N_CORES)),
                               **_CACHE.get("run_kwargs", {}))
    _CACHE["last"] = res
    out = np.concatenate([np.asarray(res.results[c]["out"]) for c in range(N_CORES)],
                         axis=0)
    return out.astype(np.float32)


# revision 22
# speedup vs baseline: 1.4887x; 1.1561x over previous
"""Trainium2 Bass kernel for nn_DiscriminatorBlock_38878043963811.

Strategy (v2)
-------------
Data-parallel over batch: 16 images -> 8 cores x 2 images. No collectives.

Algebraic restructuring (exact up to bf16, host-side folds):
  The ENTIRE linear path (fromrgb 1x1 + vertical & horizontal depthwise +
  low-rank residual + point 1x1) is one 3x3x3->512 conv = a single K=27
  matmul, M27 host-composed (clamp provably inactive, biases zero).

  TRANSPOSED execution: for each image row h, one matmul produces
  z^T[h] = [128 w, 512 c] (lhsT = shifted-sin patch [27, 128w] slice,
  rhs = M27 [27, 512c]).  PSUM is evacuated by ACT (Prelu, gain+FIR norms
  folded into scale) -- with a fraction of row-pairs offloaded to DVE as
  2-op prelu (max/scale + scalar_tensor_tensor) to balance engines.

  The whole 2x2-downsampling FIR ([1,3,3,1]x[1,3,3,1], stride 2) then runs
  on the PE as 4 tiny accumulating matmuls per output tile: for each
  (img, ho, ch-block): out2[128c, 64wo] += zT[2ho-1+a] (stationary) @
  (f_a * Fh) [128w, 64wo] (moving, only 64 columns -> 26.7ns each).
  out2 accumulates in PSUM f32, is evacuated to SBUF (DVE copy), then DMA'd.
"""

import sys

sys.path.insert(0, "/opt/trn_rl_repo")

import numpy as np
import ml_dtypes

import concourse.bass as bass
import concourse.bacc as bacc
import concourse.tile as tile
from concourse import mybir
from concourse.bass_utils import run_bass_kernel_spmd

f32 = mybir.dt.float32
bf16 = mybir.dt.bfloat16
AF = mybir.ActivationFunctionType
ALU = mybir.AluOpType

# ---- problem constants (hardcoded; kernel.py must be self-contained) ----
B, IMG_C, IN_C, OUT_C, S = 16, 3, 256, 512, 128
HIDDEN = IN_C
KGEN_IN = 32
KSIZE = 3
N_CORES = 8
B_LOC = B // N_CORES            # 2 images per core
GDW = np.float32(1.0 / np.sqrt(KSIZE))
ACT_SCALE = float(np.sqrt(2.0) / 64.0)   # lrelu gain * both FIR /8 norms

SFLAT = 130 * 130               # padded image, flattened with pitch 130
CH_Z = 32                       # z rows per s-chunk
SROWS = CH_Z + 2                # image rows held per chunk (3-tap halo)
CHW = SROWS * 130               # flat elems per s chunk

DVE_EVERY = 5                   # every 5th z row-pair prelu'd on DVE (~20%)

_CACHE = {}


def _sample_weight_np(grid, coeff, gauss_sigma, gauss_x, low_filter):
    """numpy port of reference._sample_weight (fp32)."""
    basis = np.sin(grid * np.float32(2.0 * np.pi)) * np.float32(np.exp(-0.5))
    w = coeff @ basis / np.float32(np.sqrt(HIDDEN))
    w = w - w.mean(dtype=np.float32)
    w = w * (1.0 / np.sqrt(np.mean(w * w, axis=0, keepdims=True, dtype=np.float32) + 1e-8))
    gs = 1.0 + gauss_sigma ** 2 / 5.0
    w = (w * np.exp(-(gauss_x ** 2) / (2.0 * gs))).astype(np.float32)
    nt = low_filter.shape[0]
    T = w.shape[1] - nt + 1
    out = np.empty((w.shape[0], T), np.float32)
    for t in range(T):
        out[:, t] = (w[:, t : t + nt] * low_filter[None, :]).sum(axis=1)
    return out[:, ::2]


def _build_program():
    nc = bacc.Bacc(None, target_bir_lowering=False)
    s_d = nc.declare_dram_parameter("s", [B_LOC, 27, SFLAT], bf16, isOutput=False)
    m27_d = nc.declare_dram_parameter("m27", [27, OUT_C], bf16, isOutput=False)
    fh_d = nc.declare_dram_parameter("fh", [128, 256], bf16, isOutput=False)
    out_d = nc.declare_dram_parameter("out", [B_LOC, OUT_C, 64, 64], bf16, isOutput=True)

    with tile.TileContext(nc) as tc:
        with (
            tc.tile_pool(name="const", bufs=1) as cpool,
            tc.tile_pool(name="spool", bufs=5) as spool,
            tc.tile_pool(name="zsb", bufs=8) as zpool,
            tc.tile_pool(name="qsc", bufs=2) as qpool,
            tc.tile_pool(name="osb", bufs=6) as opool,
            tc.tile_pool(name="zps", bufs=3, space="PSUM") as zps,
            tc.tile_pool(name="ops", bufs=2, space="PSUM") as ops,
        ):
            m27t = cpool.tile([27, OUT_C], bf16)
            nc.sync.dma_start(m27t[:], m27_d[:])
            fht = cpool.tile([128, 256], bf16)
            nc.sync.dma_start(fht[:], fh_d[:])

            pair_idx = 0  # global z row-pair counter (for DVE routing)

            for b in range(B_LOC):
                # s chunks for this image (4 chunks of 34 rows, stride 32)
                schunks = []
                for j in range(4):
                    st = spool.tile([27, CHW], bf16, tag="s", name=f"s_{b}_{j}")
                    if b == 0 and j == 0:
                        # split the first load so PE can start sooner
                        nc.sync.dma_start(st[:, :1300], s_d[b, :, :1300])
                        nc.sync.dma_start(st[:, 1300:], s_d[b, :, 1300:CHW])
                    else:
                        nc.sync.dma_start(
                            st[:], s_d[b, :, CH_Z * 130 * j : CH_Z * 130 * j + CHW]
                        )
                    schunks.append(st)

                zrows = [None] * 32        # 4-row SBUF group tiles
                pending = []               # ready FIR quanta (octet, cb)
                next_o = 0                 # next octet awaiting readiness

                def fir_quantum(o, cb, b=b, zrows=zrows):
                    """FIR matmuls for octet o (8 ho) x one ch-block; then
                    evac to SBUF bf16 + DMA."""
                    op = ops.tile([128, 512], f32, tag="ops",
                                  name=f"op_{b}_{o}_{cb}")
                    for ho in range(8 * o, 8 * o + 8):
                        taps = [(a, 2 * ho - 1 + a) for a in range(4)
                                if 0 <= 2 * ho - 1 + a <= 127]
                        for i, (a, row) in enumerate(taps):
                            g2, sl = row // 4, row % 4
                            lhsT = zrows[g2][:, sl * 512 + cb * 128:
                                             sl * 512 + cb * 128 + 128]
                            nc.tensor.matmul(
                                op[:, (ho % 8) * 64 : (ho % 8) * 64 + 64],
                                lhsT, fht[:, a * 64 : a * 64 + 64],
                                start=(i == 0), stop=(i == len(taps) - 1),
                            )
                    ot = opool.tile([128, 512], bf16, tag="osb",
                                    name=f"ot_{b}_{o}_{cb}")
                    nc.vector.tensor_copy(ot[:], op[:])
                    nc.sync.dma_start(
                        out_d[b, cb * 128 : cb * 128 + 128,
                              8 * o : 8 * o + 8, :],
                        ot[:].rearrange("p (r w) -> p r w", w=64),
                    )

                for g in range(32):       # 4-row z groups
                    zt = zpool.tile([128, 4 * 512], bf16, tag="zsb",
                                    name=f"z_{b}_{g}")
                    zrows[g] = zt
                    j = g // 8            # s chunk index
                    for half in range(2):
                        zp = zps.tile([128, 1024], f32, tag="zps",
                                      name=f"zp_{b}_{g}_{half}")
                        for r2 in range(2):
                            h = 4 * g + 2 * half + r2
                            lhsT = schunks[j][:, (h - CH_Z * j) * 130:
                                              (h - CH_Z * j) * 130 + 128]
                            nc.tensor.matmul(zp[:, r2 * 512 : r2 * 512 + 512],
                                             lhsT, m27t[:],
                                             start=True, stop=True)
                        dst = zt[:, half * 1024 : half * 1024 + 1024]
                        if (pair_idx - b * 64) % DVE_EVERY == 2:
                            # DVE prelu: y = s*x (1x from PSUM), then in
                            # bf16 SBUF fast modes: prelu(y) = max(y, 0.2y)
                            c = qpool.tile([128, 1024], bf16, tag="qc",
                                           name=f"c_{b}_{g}_{half}")
                            q = qpool.tile([128, 1024], bf16, tag="qq",
                                           name=f"q_{b}_{g}_{half}")
                            nc.vector.tensor_scalar(
                                c[:], zp[:], ACT_SCALE, None, ALU.mult)
                            nc.vector.tensor_scalar(
                                q[:], c[:], 0.2, None, ALU.mult)
                            nc.vector.tensor_tensor(dst, c[:], q[:], ALU.max)
                        else:
                            nc.scalar.activation(dst, zp[:], AF.Prelu,
                                                 bias=0.0, scale=ACT_SCALE,
                                                 alpha=0.2)
                        pair_idx += 1
                        # octet o's z window completes at row 16o+16
                        # (octet 7 at row 127 -- its row-128 tap is skipped)
                        rows_done = 4 * g + 2 * half + 1
                        if next_o <= 7 and rows_done >= (
                                16 * next_o + 16 if next_o < 7 else 127):
                            pending.extend((pair_idx, next_o, cb)
                                           for cb in range(4))
                            next_o += 1
                        # pop one quantum, with two pairs of lead time so
                        # its last z evac (ACT or DVE+Pool) has finished
                        if pending and pending[0][0] < pair_idx - 1:
                            _, o_, cb_ = pending.pop(0)
                            fir_quantum(o_, cb_)
                while pending:
                    _, o_, cb_ = pending.pop(0)
                    fir_quantum(o_, cb_)

    nc.compile()
    return nc


def kernel(**inputs):
    inputs = {k: np.asarray(v) for k, v in inputs.items()}
    img = inputs["img"].astype(np.float32)
    assert img.shape == (B, IMG_C, S, S)

    # ---- host-side weight generation (tiny) ----
    freqs = inputs["freqs"].astype(np.float32)
    phases = inputs["phases"].astype(np.float32)
    g = ((np.arange(KGEN_IN, dtype=np.float32) - (KGEN_IN - 1) / 2.0)
         * np.float32(2.0 / (KGEN_IN + 1)))
    gsig = np.float32(inputs["gauss_sigma"])
    gx = inputs["gauss_x"].astype(np.float32)
    lf = inputs["low_filter"].astype(np.float32)
    hz = _sample_weight_np(freqs[:, 0:1] * g[None, :] + phases[:, None],
                           inputs["hz_outdim"].astype(np.float32), gsig, gx, lf)
    vt = _sample_weight_np(freqs[:, 1:2] * g[None, :] + phases[:, None],
                           inputs["vt_outdim"].astype(np.float32), gsig, gx, lf)

    Wfr = inputs["fromrgb_w"][:, :, 0, 0].astype(np.float32) * np.float32(1.0 / np.sqrt(IMG_C))
    assert np.abs(Wfr).sum(1).max() < 250.0, "fromrgb clamp would be active"
    assert np.all(inputs["fromrgb_b"] == 0.0), "nonzero fromrgb bias unsupported"
    assert np.all(inputs["point_b"] == 0.0), "nonzero point bias unsupported"

    # k27[(d*3+jj)*3+r, c] = vt[c,d]*hz[c,jj]*GDW^2*Wfr[c,r]
    k9_np = np.zeros((27, IN_C), np.float32)
    for d in range(3):
        for r in range(3):
            for jj in range(3):
                k9_np[(d * 3 + jj) * 3 + r, :] = (
                    vt[:, d] * hz[:, jj] * GDW * GDW * Wfr[:, r]
                )
    L = inputs["lr_weight0"][:, :, 0, 0].astype(np.float32) * np.float32(1.0 / np.sqrt(IN_C))
    Pp = inputs["point_w"][:, :, 0, 0].astype(np.float32) * np.float32(1.0 / np.sqrt(IN_C))
    plw3 = (Pp @ L @ Wfr).T                      # [3, 512]
    m27_np = k9_np @ Pp.T                        # [27, 512]
    for r in range(3):
        m27_np[12 + r] += plw3[r]

    # FIR moving matrices: fh[w, a*64+wo] = f_a * f_b, b = w-(2wo-1)
    fir4 = np.array([1.0, 3.0, 3.0, 1.0], np.float32)
    fh_np = np.zeros((128, 256), np.float32)
    for a in range(4):
        for wo in range(64):
            for bb in range(4):
                w = 2 * wo - 1 + bb
                if 0 <= w <= 127:
                    fh_np[w, a * 64 + wo] = fir4[a] * fir4[bb]

    # shifted-sin flat layout: s27[b, (d*3+jj)*3+r, i] = spad[b, r, i + d*130 + jj]
    spad = np.zeros((B, IMG_C, 130, 130), np.float32)
    spad[:, :, 1:129, 1:129] = np.sin(img)
    spadf = np.zeros((B, IMG_C, SFLAT + 262), np.float32)
    spadf[:, :, :SFLAT] = spad.reshape(B, IMG_C, -1)
    s27_np = np.empty((B, 27, SFLAT), np.float32)
    for d in range(3):
        for jj in range(3):
            off = d * 130 + jj
            for r in range(3):
                s27_np[:, (d * 3 + jj) * 3 + r, :] = spadf[:, r, off : off + SFLAT]
    s27_np = s27_np.astype(ml_dtypes.bfloat16)

    shared = dict(
        m27=m27_np.astype(ml_dtypes.bfloat16),
        fh=fh_np.astype(ml_dtypes.bfloat16),
    )
    in_maps = [dict(s=np.ascontiguousarray(s27_np[c * B_LOC : (c + 1) * B_LOC]), **shared)
               for c in range(N_CORES)]

    if "nc" not in _CACHE:
        _CACHE["nc"] = _build_program()
    res = run_bass_kernel_spmd(_CACHE["nc"], in_maps, list(range(N_CORES)),
                               **_CACHE.get("run_kwargs", {}))
    _CACHE["last"] = res
    out = np.concatenate([np.asarray(res.results[c]["out"]) for c in range(N_CORES)],
                         axis=0)
    return out.astype(np.float32)


# revision 25
# speedup vs baseline: 1.5023x; 1.0091x over previous
"""Trainium2 Bass kernel for nn_DiscriminatorBlock_38878043963811.

Strategy (v2)
-------------
Data-parallel over batch: 16 images -> 8 cores x 2 images. No collectives.

Algebraic restructuring (exact up to bf16, host-side folds):
  The ENTIRE linear path (fromrgb 1x1 + vertical & horizontal depthwise +
  low-rank residual + point 1x1) is one 3x3x3->512 conv = a single K=27
  matmul, M27 host-composed (clamp provably inactive, biases zero).

  TRANSPOSED execution: for each image row h, one matmul produces
  z^T[h] = [128 w, 512 c] (lhsT = shifted-sin patch [27, 128w] slice,
  rhs = M27 [27, 512c]).  PSUM is evacuated by ACT (Prelu, gain+FIR norms
  folded into scale) -- with a fraction of row-pairs offloaded to DVE as
  2-op prelu (max/scale + scalar_tensor_tensor) to balance engines.

  The whole 2x2-downsampling FIR ([1,3,3,1]x[1,3,3,1], stride 2) then runs
  on the PE as 4 tiny accumulating matmuls per output tile: for each
  (img, ho, ch-block): out2[128c, 64wo] += zT[2ho-1+a] (stationary) @
  (f_a * Fh) [128w, 64wo] (moving, only 64 columns -> 26.7ns each).
  out2 accumulates in PSUM f32, is evacuated to SBUF (DVE copy), then DMA'd.
"""

import sys

sys.path.insert(0, "/opt/trn_rl_repo")

import numpy as np
import ml_dtypes

import concourse.bass as bass
import concourse.bacc as bacc
import concourse.tile as tile
from concourse import mybir
from concourse.bass_utils import run_bass_kernel_spmd

f32 = mybir.dt.float32
bf16 = mybir.dt.bfloat16
AF = mybir.ActivationFunctionType
ALU = mybir.AluOpType

# ---- problem constants (hardcoded; kernel.py must be self-contained) ----
B, IMG_C, IN_C, OUT_C, S = 16, 3, 256, 512, 128
HIDDEN = IN_C
KGEN_IN = 32
KSIZE = 3
N_CORES = 8
B_LOC = B // N_CORES            # 2 images per core
GDW = np.float32(1.0 / np.sqrt(KSIZE))
ACT_SCALE = float(np.sqrt(2.0) / 64.0)   # lrelu gain * both FIR /8 norms

SFLAT = 130 * 130               # padded image, flattened with pitch 130
CH_Z = 32                       # z rows per s-chunk
SROWS = CH_Z + 2                # image rows held per chunk (3-tap halo)
CHW = SROWS * 130               # flat elems per s chunk

DVE_EVERY = 5                   # every 5th z row-pair prelu'd on DVE (~20%)

_CACHE = {}


def _sample_weight_np(grid, coeff, gauss_sigma, gauss_x, low_filter):
    """numpy port of reference._sample_weight (fp32)."""
    basis = np.sin(grid * np.float32(2.0 * np.pi)) * np.float32(np.exp(-0.5))
    w = coeff @ basis / np.float32(np.sqrt(HIDDEN))
    w = w - w.mean(dtype=np.float32)
    w = w * (1.0 / np.sqrt(np.mean(w * w, axis=0, keepdims=True, dtype=np.float32) + 1e-8))
    gs = 1.0 + gauss_sigma ** 2 / 5.0
    w = (w * np.exp(-(gauss_x ** 2) / (2.0 * gs))).astype(np.float32)
    nt = low_filter.shape[0]
    T = w.shape[1] - nt + 1
    out = np.empty((w.shape[0], T), np.float32)
    for t in range(T):
        out[:, t] = (w[:, t : t + nt] * low_filter[None, :]).sum(axis=1)
    return out[:, ::2]


def _build_program():
    nc = bacc.Bacc(None, target_bir_lowering=False)
    s_d = nc.declare_dram_parameter("s", [B_LOC, 27, SFLAT], bf16, isOutput=False)
    m27_d = nc.declare_dram_parameter("m27", [27, OUT_C], bf16, isOutput=False)
    fh_d = nc.declare_dram_parameter("fh", [128, 256], bf16, isOutput=False)
    out_d = nc.declare_dram_parameter("out", [B_LOC, OUT_C, 64, 64], bf16, isOutput=True)

    with tile.TileContext(nc) as tc:
        with (
            tc.tile_pool(name="const", bufs=1) as cpool,
            tc.tile_pool(name="spool", bufs=5) as spool,
            tc.tile_pool(name="zsb", bufs=8) as zpool,
            tc.tile_pool(name="qsc", bufs=2) as qpool,
            tc.tile_pool(name="osb", bufs=6) as opool,
            tc.tile_pool(name="zps", bufs=3, space="PSUM") as zps,
            tc.tile_pool(name="ops", bufs=2, space="PSUM") as ops,
        ):
            m27t = cpool.tile([27, OUT_C], bf16)
            nc.scalar.dma_start(m27t[:], m27_d[:])
            fht = cpool.tile([128, 256], bf16)
            nc.scalar.dma_start(fht[:], fh_d[:])

            pair_idx = 0  # global z row-pair counter (for DVE routing)

            for b in range(B_LOC):
                # s chunks for this image (4 chunks of 34 rows, stride 32)
                schunks = []
                for j in range(4):
                    st = spool.tile([27, CHW], bf16, tag="s", name=f"s_{b}_{j}")
                    if b == 0 and j == 0:
                        # split the first load so PE can start sooner
                        nc.sync.dma_start(st[:, :1300], s_d[b, :, :1300])
                        nc.sync.dma_start(st[:, 1300:], s_d[b, :, 1300:CHW])
                    else:
                        nc.sync.dma_start(
                            st[:], s_d[b, :, CH_Z * 130 * j : CH_Z * 130 * j + CHW]
                        )
                    schunks.append(st)

                zrows = [None] * 32        # 4-row SBUF group tiles
                pending = []               # ready FIR quanta (octet, cb)
                next_o = 0                 # next octet awaiting readiness

                def fir_quantum(o, cb, split=False, b=b, zrows=zrows):
                    """FIR matmuls for octet o (8 ho) x one ch-block; then
                    evac to SBUF bf16 + DMA (in 2 halves when split, to
                    shorten the end-of-kernel tail)."""
                    op = ops.tile([128, 512], f32, tag="ops",
                                  name=f"op_{b}_{o}_{cb}")
                    ot = opool.tile([128, 512], bf16, tag="osb",
                                    name=f"ot_{b}_{o}_{cb}")
                    parts = ((0, 4), (4, 8)) if split else ((0, 8),)
                    for lo, hi in parts:
                        for ho in range(8 * o + lo, 8 * o + hi):
                            taps = [(a, 2 * ho - 1 + a) for a in range(4)
                                    if 0 <= 2 * ho - 1 + a <= 127]
                            for i, (a, row) in enumerate(taps):
                                g2, sl = row // 4, row % 4
                                lhsT = zrows[g2][:, sl * 512 + cb * 128:
                                                 sl * 512 + cb * 128 + 128]
                                nc.tensor.matmul(
                                    op[:, (ho % 8) * 64 : (ho % 8) * 64 + 64],
                                    lhsT, fht[:, a * 64 : a * 64 + 64],
                                    start=(i == 0), stop=(i == len(taps) - 1),
                                )
                        nc.vector.tensor_copy(ot[:, lo * 64 : hi * 64],
                                              op[:, lo * 64 : hi * 64])
                        nc.sync.dma_start(
                            out_d[b, cb * 128 : cb * 128 + 128,
                                  8 * o + lo : 8 * o + hi, :],
                            ot[:, lo * 64 : hi * 64].rearrange(
                                "p (r w) -> p r w", w=64),
                        )

                for g in range(32):       # 4-row z groups
                    zt = zpool.tile([128, 4 * 512], bf16, tag="zsb",
                                    name=f"z_{b}_{g}")
                    zrows[g] = zt
                    j = g // 8            # s chunk index
                    for half in range(2):
                        zp = zps.tile([128, 1024], f32, tag="zps",
                                      name=f"zp_{b}_{g}_{half}")
                        for r2 in range(2):
                            h = 4 * g + 2 * half + r2
                            lhsT = schunks[j][:, (h - CH_Z * j) * 130:
                                              (h - CH_Z * j) * 130 + 128]
                            nc.tensor.matmul(zp[:, r2 * 512 : r2 * 512 + 512],
                                             lhsT, m27t[:],
                                             start=True, stop=True)
                        dst = zt[:, half * 1024 : half * 1024 + 1024]
                        if (pair_idx - b * 64) % DVE_EVERY == 2:
                            # DVE prelu: y = s*x (1x from PSUM), then in
                            # bf16 SBUF fast modes: prelu(y) = max(y, 0.2y)
                            c = qpool.tile([128, 1024], bf16, tag="qc",
                                           name=f"c_{b}_{g}_{half}")
                            q = qpool.tile([128, 1024], bf16, tag="qq",
                                           name=f"q_{b}_{g}_{half}")
                            nc.vector.tensor_scalar(
                                c[:], zp[:], ACT_SCALE, None, ALU.mult)
                            nc.vector.tensor_scalar(
                                q[:], c[:], 0.2, None, ALU.mult)
                            nc.vector.tensor_tensor(dst, c[:], q[:], ALU.max)
                        else:
                            nc.scalar.activation(dst, zp[:], AF.Prelu,
                                                 bias=0.0, scale=ACT_SCALE,
                                                 alpha=0.2)
                        pair_idx += 1
                        # octet o's z window completes at row 16o+16
                        # (octet 7 at row 127 -- its row-128 tap is skipped)
                        rows_done = 4 * g + 2 * half + 1
                        if next_o <= 7 and rows_done >= (
                                16 * next_o + 16 if next_o < 7 else 127):
                            pending.extend((pair_idx, next_o, cb)
                                           for cb in range(4))
                            next_o += 1
                        # pop one quantum, with two pairs of lead time so
                        # its last z evac (ACT or DVE+Pool) has finished
                        if pending and pending[0][0] < pair_idx - 1:
                            _, o_, cb_ = pending.pop(0)
                            fir_quantum(o_, cb_)
                while pending:
                    _, o_, cb_ = pending.pop(0)
                    fir_quantum(o_, cb_, split=(b == B_LOC - 1
                                                and not pending))

    nc.compile()
    return nc


def kernel(**inputs):
    inputs = {k: np.asarray(v) for k, v in inputs.items()}
    img = inputs["img"].astype(np.float32)
    assert img.shape == (B, IMG_C, S, S)

    # ---- host-side weight generation (tiny) ----
    freqs = inputs["freqs"].astype(np.float32)
    phases = inputs["phases"].astype(np.float32)
    g = ((np.arange(KGEN_IN, dtype=np.float32) - (KGEN_IN - 1) / 2.0)
         * np.float32(2.0 / (KGEN_IN + 1)))
    gsig = np.float32(inputs["gauss_sigma"])
    gx = inputs["gauss_x"].astype(np.float32)
    lf = inputs["low_filter"].astype(np.float32)
    hz = _sample_weight_np(freqs[:, 0:1] * g[None, :] + phases[:, None],
                           inputs["hz_outdim"].astype(np.float32), gsig, gx, lf)
    vt = _sample_weight_np(freqs[:, 1:2] * g[None, :] + phases[:, None],
                           inputs["vt_outdim"].astype(np.float32), gsig, gx, lf)

    Wfr = inputs["fromrgb_w"][:, :, 0, 0].astype(np.float32) * np.float32(1.0 / np.sqrt(IMG_C))
    assert np.abs(Wfr).sum(1).max() < 250.0, "fromrgb clamp would be active"
    assert np.all(inputs["fromrgb_b"] == 0.0), "nonzero fromrgb bias unsupported"
    assert np.all(inputs["point_b"] == 0.0), "nonzero point bias unsupported"

    # k27[(d*3+jj)*3+r, c] = vt[c,d]*hz[c,jj]*GDW^2*Wfr[c,r]
    k9_np = np.zeros((27, IN_C), np.float32)
    for d in range(3):
        for r in range(3):
            for jj in range(3):
                k9_np[(d * 3 + jj) * 3 + r, :] = (
                    vt[:, d] * hz[:, jj] * GDW * GDW * Wfr[:, r]
                )
    L = inputs["lr_weight0"][:, :, 0, 0].astype(np.float32) * np.float32(1.0 / np.sqrt(IN_C))
    Pp = inputs["point_w"][:, :, 0, 0].astype(np.float32) * np.float32(1.0 / np.sqrt(IN_C))
    plw3 = (Pp @ L @ Wfr).T                      # [3, 512]
    m27_np = k9_np @ Pp.T                        # [27, 512]
    for r in range(3):
        m27_np[12 + r] += plw3[r]

    # FIR moving matrices: fh[w, a*64+wo] = f_a * f_b, b = w-(2wo-1)
    fir4 = np.array([1.0, 3.0, 3.0, 1.0], np.float32)
    fh_np = np.zeros((128, 256), np.float32)
    for a in range(4):
        for wo in range(64):
            for bb in range(4):
                w = 2 * wo - 1 + bb
                if 0 <= w <= 127:
                    fh_np[w, a * 64 + wo] = fir4[a] * fir4[bb]

    # shifted-sin flat layout: s27[b, (d*3+jj)*3+r, i] = spad[b, r, i + d*130 + jj]
    spad = np.zeros((B, IMG_C, 130, 130), np.float32)
    spad[:, :, 1:129, 1:129] = np.sin(img)
    spadf = np.zeros((B, IMG_C, SFLAT + 262), np.float32)
    spadf[:, :, :SFLAT] = spad.reshape(B, IMG_C, -1)
    s27_np = np.empty((B, 27, SFLAT), np.float32)
    for d in range(3):
        for jj in range(3):
            off = d * 130 + jj
            for r in range(3):
                s27_np[:, (d * 3 + jj) * 3 + r, :] = spadf[:, r, off : off + SFLAT]
    s27_np = s27_np.astype(ml_dtypes.bfloat16)

    shared = dict(
        m27=m27_np.astype(ml_dtypes.bfloat16),
        fh=fh_np.astype(ml_dtypes.bfloat16),
    )
    in_maps = [dict(s=np.ascontiguousarray(s27_np[c * B_LOC : (c + 1) * B_LOC]), **shared)
               for c in range(N_CORES)]

    if "nc" not in _CACHE:
        _CACHE["nc"] = _build_program()
    res = run_bass_kernel_spmd(_CACHE["nc"], in_maps, list(range(N_CORES)),
                               **_CACHE.get("run_kwargs", {}))
    _CACHE["last"] = res
    out = np.concatenate([np.asarray(res.results[c]["out"]) for c in range(N_CORES)],
                         axis=0)
    return out.astype(np.float32)
